# revision 40
# baseline (speedup 1.0000x reference)
"""Trainium2 Bass kernel for nn_Encoder_z0_ODE_RNN (ODE-RNN encoder).

Data-parallel over batch (512 -> 8 cores x 64), transposed on-device layout
(feature dim on SBUF partitions, batch in the free dim) so the recurrence
needs no on-device transposes.

v3 (default): the input GEMM (W_ih @ data, gate bias folded) is computed in
512-wide bulk quanta interleaved into the recurrence's PE wait gaps, with
junk matmuls padding the rest so the tensor engine never idles (idle
triggers a ~13.7us half-clock p-state).  Per step: ODE Euler update with dt
folded into tanh(u) before the second GEMM, LSTM gates via 64 small matmuls
accumulating in PSUM, pre-add + activations + cell update on DVE/Act/GpSimd
in the full-width 128-partition layout.  Fast path assumes every (t,b) has
at least one observed feature (holds w.p. 1 - T*B*2^-256 for the random
mask; checked on host) so the observation merge is the identity.

v1 (fallback for nonzero ODE biases or unobserved rows): DRAM-staged input
precompute + masked merge via copy_predicated.
"""

import numpy as np
import ml_dtypes

import concourse.bass as bass
import concourse.mybir as mybir
import concourse.tile as tile
from concourse.bass_utils import run_bass_kernel_spmd

dt = mybir.dt
F32, BF16 = dt.float32, dt.bfloat16
AF = mybir.ActivationFunctionType
BF = ml_dtypes.bfloat16

B, T, D, L, H, Z0 = 512, 128, 256, 512, 512, 512
NCORES = 8
BS = B // NCORES            # 64 batch rows per core
TB = T * BS                 # 8192 columns of dataT per core
KL = L // 128               # 4 k-tiles over L
KD = (2 * D) // 128         # 4 k-tiles over 2D
JG = (4 * L) // 128         # 16 j-tiles over gates
C = 64                      # batch columns per L-chunk (transposed layout)

RING = 24          # pre ring depth in steps (3 chunks of 8)
CS = 8             # steps per bulk chunk (512-wide moving operand)
NCH = T // CS      # 16 bulk chunks
# fill budgets per PE wait site, in ~213ns units (1 unit = one 512w matmul)
FILL_B = 3         # between u and v (tanh + dt-mul gap)
FILL_C = 3         # between v and gates (xobf gap)
FILL_D = 8         # after gates (elementwise tail gap)


def _patch_tile_drain(max_waits=1):
    """walrus rejects >~4 sync waits on a single instruction; Tile's tail
    drain aggregates every outstanding semaphore. Split across drains."""
    from concourse.vector_clock import ScopedClock

    def _drain_and_barrier(self, tick_clock, wait_clock):
        drain_inst = self.nc.sync.drain()
        wait_clock.add_sem_waits(drain_inst.ins,
                                 ScopedClock({None: tick_clock.global_clock}))
        si = drain_inst.ins.sync_info
        if si is not None and len(si.on_wait) > max_waits:
            waits = list(si.on_wait)
            drain_inst.ins.sync_info = mybir.SyncInfo(
                on_wait=waits[:max_waits], on_update=list(si.on_update))
            rest = waits[max_waits:]
            while rest:
                extra = self.nc.sync.drain()
                extra.ins.sync_info = mybir.SyncInfo(
                    on_wait=rest[:max_waits], on_update=[])
                rest = rest[max_waits:]
        self.nc.all_engine_barrier()
        assert self.sems is not None
        popped = self.nc._tile_sem_poison_stack.pop()
        assert popped is self._sem_poison
        self.nc.clear_and_free_semaphores(list(self.sems.allocated().values()))
        self.nc.all_engine_barrier()

    tile.TileContext._drain_and_barrier = _drain_and_barrier


def _legalize_waits(nc, default_limit=1):
    """walrus per-instruction sync-wait slots are tiny. Move excess waits
    onto same-engine NOPs inserted right before the offending instruction."""
    ctr = [0]
    for f in nc.m.functions:
        for b in f.blocks:
            il = b.instructions
            i = 0
            while i < len(il):
                inst = il[i]
                si = inst.sync_info
                if si is not None:
                    waits = list(si.on_wait)
                    if len(waits) > default_limit:
                        excess, keep = waits[:-default_limit], waits[-default_limit:]
                        inst.sync_info = mybir.SyncInfo(
                            on_wait=keep, on_update=list(si.on_update))
                        for w in excess:
                            ctr[0] += 1
                            nop = mybir.InstNoOp(
                                name=f"waitnop{ctr[0]}", ins=[], outs=[])
                            nop.engine = inst.engine
                            nop.sync_info = mybir.SyncInfo(
                                on_wait=[w], on_update=[])
                            il.insert(i, nop)
                            i += 1
                i += 1


# ============================ v3 build ====================================

class _BulkFeeder:
    """Emits the input GEMM in 512-wide quanta (one j-tile x 4 k matmuls +
    biased evict to the pre ring) at PE wait gaps; pads with junk matmuls
    so the tensor engine never idles."""

    def __init__(self, nc, bpool, junk_ps, wihv, data_sb, junk_mov, pre_view,
                 bg_sb):
        self.nc = nc
        self.bpool = bpool
        self.junk_ps = junk_ps
        self.wihv = wihv
        self.data_sb = data_sb
        self.junk_mov = junk_mov
        self.pre_view = pre_view
        self.bg_sb = bg_sb
        self.pos = 0            # quanta emitted (= chunk * JG + j)
        self.limit = 0
        self.njunk = 0
        self.pending = []       # quanta awaiting evict emission
        self.step_quanta = 0
        self.warmup = False

    def allow(self, n_chunks):
        self.limit = min(n_chunks, NCH) * JG
        self.step_quanta = 0

    def _emit_quantum(self):
        c, j = divmod(self.pos, JG)
        nc = self.nc
        if self.warmup and self.pos % 2 == 1:
            bp = self.junk_ps   # junk bank is free pre-recurrence
        else:
            bp = self.bpool.tile([128, CS * C], F32, tag="bulk",
                                 name=f"blk{c}_{j}")
        for k in range(KD):
            nc.tensor.matmul(
                bp[:], self.wihv(k, j),
                self.data_sb[:, k * TB + c * CS * C:k * TB + (c + 1) * CS * C],
                start=(k == 0), stop=(k == KD - 1))
        self.pending.append((c, j, bp))
        self.pos += 1
        self.step_quanta += 1

    def emit_evict(self, eng):
        """Evict one pending bulk quantum to the pre ring (bias folded).
        Called at queue positions where the engine is naturally idle so the
        evict never delays critical tail ops."""
        if not self.pending:
            return
        c, j, bp = self.pending.pop(0)
        s0 = (c * CS) % RING
        out = self.pre_view[:, s0:s0 + CS, j, :]
        if eng == "act":
            self.nc.scalar.activation(out, bp[:], AF.Identity,
                                      bias=self.bg_sb[:, j:j + 1])
        else:
            self.nc.vector.tensor_scalar_add(out, bp[:], self.bg_sb[:, j:j + 1])

    def _emit_junk(self):
        self.njunk += 1
        self.nc.tensor.matmul(self.junk_ps[:], self.wihv(0, 0),
                              self.junk_mov, start=True, stop=True,
                              skip_group_check=True)

    def fill(self, units, max_quanta=2):
        done = 0
        while (done + 4 <= units and self.pos < self.limit
               and self.step_quanta < max_quanta
               and len(self.pending) < 2):
            self._emit_quantum()
            done += 4
        while done < units:
            self._emit_junk()
            done += 1

    def drain_chunks(self, n_chunks):
        self.warmup = True
        self.allow(n_chunks)
        evict = 0
        while self.pos < self.limit:
            self._emit_quantum()
            if len(self.pending) >= 2:
                self.emit_evict(["vec", "act"][evict % 2])
                evict += 1
        while self.pending:
            self.emit_evict(["vec", "act"][evict % 2])
            evict += 1
        self.warmup = False


def _build_v3(legalize=True):
    _patch_tile_drain()
    nc = bass.Bass()
    g = lambda n, s, d: nc.dram_tensor(n, s, d, kind="ExternalInput").ap()
    dataT_d = g("dataT", [KD, 128, TB], BF16)
    dtm_d = g("dtm", [T, 128, 512], F32)
    wo1_d = g("wo1", [KL, KL, 128, 128], BF16)
    wo2_d = g("wo2", [KL, KL, 128, 128], BF16)
    whh_d = g("whh", [KL, JG, 128, 128], BF16)
    wih_d = g("wih", [KD, JG, 128, 128], BF16)
    bg_d = g("bg", [128, JG], F32)
    wt1_d = g("wt1", [KL, 128, 100], BF16)
    wt2_d = g("wt2", [8, 100, 128], BF16)
    bt1_d = g("bt1t", [128, 1], F32)
    bt2_d = g("bt2t", [128, 8], F32)
    eye_d = g("eye128", [128, 128], BF16)
    mean_d = nc.dram_tensor("meanT", [KL, 128, BS], F32, kind="ExternalOutput").ap()
    std_d = nc.dram_tensor("stdT", [KL, 128, BS], F32, kind="ExternalOutput").ap()

    with tile.TileContext(nc) as tc:
        import contextlib
        with contextlib.ExitStack() as ctx:
            wpool = ctx.enter_context(tc.tile_pool(name="weights", bufs=1))
            spool = ctx.enter_context(tc.tile_pool(name="state", bufs=1))
            dpool = ctx.enter_context(tc.tile_pool(name="stream", bufs=4))
            kpool = ctx.enter_context(tc.tile_pool(name="work", bufs=2))

            # ---- resident loads (weights first; data streamed per-chunk so
            # the bulk warmup starts as soon as wih + chunk 0 land) ---------
            wih_sb = wpool.tile([128, KD * JG * 128], BF16, tag="wih")
            nc.sync.dma_start(
                wih_sb[:].rearrange("p (k j n) -> p k j n", k=KD, j=JG),
                wih_d[:].transpose([2, 0, 1, 3]))
            wo1_sb = wpool.tile([128, KL * KL * 128], BF16, tag="wo1")
            nc.sync.dma_start(
                wo1_sb[:].rearrange("p (k j n) -> p k j n", k=KL, j=KL),
                wo1_d[:].transpose([2, 0, 1, 3]))
            wo2_sb = wpool.tile([128, KL * KL * 128], BF16, tag="wo2")
            nc.sync.dma_start(
                wo2_sb[:].rearrange("p (k j n) -> p k j n", k=KL, j=KL),
                wo2_d[:].transpose([2, 0, 1, 3]))
            whh_sb = wpool.tile([128, KL * JG * 128], BF16, tag="whh")
            nc.sync.dma_start(
                whh_sb[:].rearrange("p (k j n) -> p k j n", k=KL, j=JG),
                whh_d[:].transpose([2, 0, 1, 3]))
            bg_sb = wpool.tile([128, JG], F32, tag="bg")
            nc.sync.dma_start(bg_sb[:], bg_d[:])
            wt1_sb = wpool.tile([128, KL * 100], BF16, tag="wt1")
            nc.sync.dma_start(
                wt1_sb[:].rearrange("p (k n) -> p k n", k=KL),
                wt1_d[:].transpose([1, 0, 2]))
            wt2_sb = wpool.tile([100, 8 * 128], BF16, tag="wt2")
            nc.sync.dma_start(
                wt2_sb[:].rearrange("p (j n) -> p j n", j=8),
                wt2_d[:].transpose([1, 0, 2]))
            bt1_sb = wpool.tile([128, 1], F32, tag="bt1")
            nc.sync.dma_start(bt1_sb[:], bt1_d[:])
            bt2_sb = wpool.tile([128, 8], F32, tag="bt2")
            nc.sync.dma_start(bt2_sb[:], bt2_d[:])
            eye_sb = wpool.tile([128, 128], BF16, tag="eye")
            nc.sync.dma_start(eye_sb[:], eye_d[:])

            data_sb = wpool.tile([128, KD * TB], BF16, tag="data")
            data_view = data_sb[:].rearrange("p (k n) -> p k n", k=KD)

            def dma_chunk(c):
                nc.sync.dma_start(
                    data_view[:, :, c * CS * C:(c + 1) * CS * C],
                    dataT_d[:, :, c * CS * C:(c + 1) * CS * C].transpose([1, 0, 2]))

            for c in range(3):
                dma_chunk(c)

            pre_ring = wpool.tile([128, RING * JG * C], BF16, tag="prering")
            pre_view = pre_ring[:].rearrange("p (s j c) -> p s j c",
                                             s=RING, j=JG)

            def pre_flat(s, j0, nj):
                return pre_ring[:, (s * JG + j0) * C:(s * JG + j0 + nj) * C]

            wihv = lambda k, j: wih_sb[:, (k * JG + j) * 128:(k * JG + j + 1) * 128]
            whhv = lambda k, j: whh_sb[:, (k * JG + j) * 128:(k * JG + j + 1) * 128]
            wo1v = lambda k, j: wo1_sb[:, (k * KL + j) * 128:(k * KL + j + 1) * 128]
            wo2v = lambda k, j: wo2_sb[:, (k * KL + j) * 128:(k * KL + j + 1) * 128]

            # ---- psum pools (8 banks total) --------------------------------
            odepool = ctx.enter_context(
                tc.tile_pool(name="odeps", bufs=1, space="PSUM"))
            gpool = ctx.enter_context(
                tc.tile_pool(name="gps", bufs=1, space="PSUM"))
            bpool = ctx.enter_context(
                tc.tile_pool(name="bulkps", bufs=1, space="PSUM"))
            jpool = ctx.enter_context(
                tc.tile_pool(name="junkps", bufs=1, space="PSUM"))
            junk_ps = jpool.tile([128, 512], F32, tag="junk")
            # one PSUM tile per gate function so each activation depends
            # only on its own 4 j-tiles (Tile tracks whole-tile deps)
            gp_i = gpool.tile([128, 256], F32, tag="gpi")
            gp_f = gpool.tile([128, 256], F32, tag="gpf")
            gp_o = gpool.tile([128, 256], F32, tag="gpo")
            gp_g = gpool.tile([128, 256], F32, tag="gpg")

            feeder = _BulkFeeder(nc, bpool, junk_ps, wihv, data_sb,
                                 whh_sb[:, 0:512], pre_view, bg_sb)

            # ---- states (split into L-halves so half-1 of the end-of-step
            # ladder releases next step's matmuls early) ---------------------
            cT_a = spool.tile([128, 128], F32, tag="cTa")
            cT_b = spool.tile([128, 128], F32, tag="cTb")
            xbf_a = spool.tile([128, 128], BF16, tag="xbfa")
            xbf_b = spool.tile([128, 128], BF16, tag="xbfb")
            nc.vector.memset(cT_a[:], 0.0)
            nc.vector.memset(cT_b[:], 0.0)
            nc.vector.memset(xbf_a[:], 0.0)
            nc.vector.memset(xbf_b[:], 0.0)

            def xk(k):
                """moving-operand slice for L-chunk k of the state"""
                return (xbf_a if k < 2 else xbf_b)[:, (k % 2) * C:(k % 2 + 1) * C]

            # warm-up: first two bulk chunks before the recurrence
            feeder.drain_chunks(2)

            ji, jf, jo, jg_ = 0, 4, 8, 12   # j-tile bases, gate order i,f,o,g
            for t in range(T):
                feeder.allow(t // CS + 3)
                s = t % RING
                dtm_sb = dpool.tile([128, 256], F32, tag="dtm")
                nc.sync.dma_start(dtm_sb[:], dtm_d[t][:, 0:256])
                if t % CS == 0 and t >= CS and t // CS + 2 < NCH:
                    dma_chunk(t // CS + 2)
                # evict slot A: Act is idle until tanh(u)
                feeder.emit_evict("act")

                # ODE halves: L-chunks 01 in ode_a, 23 in ode_b (separate
                # tiles so half-1 consumers don't wait on half-2 writes).
                # Each tile: cols 0:128 = u half, cols 128:256 = v half.
                ode_a = odepool.tile([128, 256], F32, tag="odea", name=f"oa{t}")
                ode_b = odepool.tile([128, 256], F32, tag="odeb", name=f"ob{t}")
                for j in range(2):
                    for k in range(KL):
                        nc.tensor.matmul(ode_a[:, j * C:(j + 1) * C],
                                         wo1v(k, j), xk(k),
                                         start=(k == 0), stop=(k == KL - 1))
                t1_a = kpool.tile([128, 128], F32, tag="t1a")
                nc.scalar.activation(t1_a[:], ode_a[:, 0:128], AF.Tanh)
                for j in range(2, KL):
                    for k in range(KL):
                        nc.tensor.matmul(ode_b[:, (j - 2) * C:(j - 1) * C],
                                         wo1v(k, j), xk(k),
                                         start=(k == 0), stop=(k == KL - 1))
                t1_b = kpool.tile([128, 128], F32, tag="t1b")
                nc.scalar.activation(t1_b[:], ode_b[:, 0:128], AF.Tanh)
                t1d_a = kpool.tile([128, 128], BF16, tag="t1da")
                nc.vector.tensor_mul(t1d_a[:], t1_a[:], dtm_sb[:, 0:128])
                t1d_b = kpool.tile([128, 128], BF16, tag="t1db")
                nc.vector.tensor_mul(t1d_b[:], t1_b[:], dtm_sb[:, 128:256])
                feeder.fill(FILL_B)
                # v-half regions are seeded with x itself (identity matmul on
                # xbf — available at step start), so the v matmuls are plain
                # accumulations and x_ode reads straight out of PSUM.  (The
                # injects carry a WAR wait on the tanh reads, covered by the
                # fill above.)
                nc.tensor.matmul(ode_a[:, 128:256], eye_sb[:], xbf_a[:],
                                 start=True, stop=False, skip_group_check=True)
                nc.tensor.matmul(ode_b[:, 128:256], eye_sb[:], xbf_b[:],
                                 start=True, stop=False, skip_group_check=True)
                # v = Wo2T @ t1d, k-halves interleaved so k01 matmuls start
                # as soon as t1d_a lands
                for j in range(KL):
                    dst = (ode_a if j < 2 else ode_b)[:, 128 + (j % 2) * C:
                                                      128 + (j % 2 + 1) * C]
                    for k in range(2):
                        nc.tensor.matmul(dst, wo2v(k, j),
                                         t1d_a[:, k * C:(k + 1) * C],
                                         start=False, stop=False,
                                         skip_group_check=True)
                for j in range(KL):
                    dst = (ode_a if j < 2 else ode_b)[:, 128 + (j % 2) * C:
                                                      128 + (j % 2 + 1) * C]
                    for k in range(2, KL):
                        nc.tensor.matmul(dst, wo2v(k, j),
                                         t1d_b[:, (k - 2) * C:(k - 1) * C],
                                         start=False, stop=(k == KL - 1),
                                         skip_group_check=True)
                feeder.fill(FILL_C)
                # x_ode is consumed only by the gates GEMM in the m==1 fast
                # path (x_{t+1} = h), so only its bf16 form is materialized.
                xobf_a = kpool.tile([128, 128], BF16, tag="xobfa")
                nc.vector.tensor_copy(xobf_a[:], ode_a[:, 128:256])
                xobf_b = kpool.tile([128, 128], BF16, tag="xobfb")
                nc.vector.tensor_copy(xobf_b[:], ode_b[:, 128:256])
                # evict slot B: DVE is idle during the gates block
                feeder.emit_evict("vec")

                # gates = pre + WhhT @ x_ode, one PSUM tile per gate function
                # (emission order f, i, g, o) so each activation starts as
                # soon as its own 4 j-tiles finish, overlapping the block.
                # pre injected with one 256-wide identity matmul per group.
                def gate_group(gp, j0, q0=0, q1=4, inject=True):
                    if inject:
                        nc.tensor.matmul(gp[:], eye_sb[:], pre_flat(s, j0, 4),
                                         start=True, stop=False,
                                         skip_group_check=True)
                    for q in range(q0, q1):
                        jj = j0 + q
                        dst = gp[:, q * C:(q + 1) * C]
                        for k in range(KL):
                            xo = xobf_a if k < 2 else xobf_b
                            nc.tensor.matmul(dst, whhv(k, jj),
                                             xo[:, (k % 2) * C:(k % 2 + 1) * C],
                                             start=False, stop=(k == KL - 1),
                                             skip_group_check=True)

                gate_group(gp_f, jf)
                sigf = kpool.tile([128, 256], F32, tag="sigf")
                nc.scalar.activation(sigf[:], gp_f[:], AF.Sigmoid)
                m1_a = kpool.tile([128, 128], F32, tag="m1a")
                nc.gpsimd.tensor_mul(m1_a[:], sigf[:, 0:128], cT_a[:])
                m1_b = kpool.tile([128, 128], F32, tag="m1b")
                nc.gpsimd.tensor_mul(m1_b[:], sigf[:, 128:256], cT_b[:])
                gate_group(gp_i, ji)
                sigi = kpool.tile([128, 256], F32, tag="sigi")
                nc.scalar.activation(sigi[:], gp_i[:], AF.Sigmoid)
                gate_group(gp_g, jg_)
                tg = kpool.tile([128, 256], F32, tag="tg")
                nc.scalar.activation(tg[:], gp_g[:], AF.Tanh)
                m2 = kpool.tile([128, 256], F32, tag="m2")
                nc.vector.tensor_mul(m2[:], sigi[:], tg[:])
                # o-group split so sigo half-1 fires mid-group; end-of-step
                # ladder (c2 -> tanh -> h) runs in halves, half-1 releasing
                # next step's u matmuls early
                gate_group(gp_o, jo, 0, 2)
                sigo_a = kpool.tile([128, 128], F32, tag="sigoa")
                nc.scalar.activation(sigo_a[:], gp_o[:, 0:128], AF.Sigmoid)
                gate_group(gp_o, jo, 2, 4, inject=False)
                feeder.fill(FILL_D)
                sigo_b = kpool.tile([128, 128], F32, tag="sigob")
                nc.scalar.activation(sigo_b[:], gp_o[:, 128:256], AF.Sigmoid)
                nc.vector.tensor_add(cT_a[:], m1_a[:], m2[:, 0:128])
                tc2_a = kpool.tile([128, 128], F32, tag="tc2a")
                nc.scalar.activation(tc2_a[:], cT_a[:], AF.Tanh)
                nc.vector.tensor_add(cT_b[:], m1_b[:], m2[:, 128:256])
                tc2_b = kpool.tile([128, 128], F32, tag="tc2b")
                nc.scalar.activation(tc2_b[:], cT_b[:], AF.Tanh)
                nc.vector.tensor_mul(xbf_a[:], sigo_a[:], tc2_a[:])
                nc.vector.tensor_mul(xbf_b[:], sigo_b[:], tc2_b[:])

            # ---- head ------------------------------------------------------
            q_ps = odepool.tile([128, 256], F32, tag="odea", name="qps")
            for k in range(KL):
                nc.tensor.matmul(q_ps[0:100, 0:C],
                                 wt1_sb[:, k * 100:(k + 1) * 100],
                                 xk(k),
                                 start=(k == 0), stop=(k == KL - 1))
            tq_bf = kpool.tile([100, C], BF16, tag="tq")
            nc.scalar.activation(tq_bf[:], q_ps[0:100, 0:C], AF.Tanh,
                                 bias=bt1_sb[0:100, :])
            for j in range(8):
                zp = gp_i if j < 4 else gp_f
                nc.tensor.matmul(zp[:, (j % 4) * C:(j % 4 + 1) * C],
                                 wt2_sb[:, j * 128:(j + 1) * 128], tq_bf[:],
                                 start=True, stop=True)
            mo = kpool.tile([128, 256], F32, tag="mo")
            so = kpool.tile([128, 256], F32, tag="so")
            for j in range(KL):
                nc.scalar.activation(mo[:, j * C:(j + 1) * C],
                                     gp_i[:, j * C:(j + 1) * C], AF.Identity,
                                     bias=bt2_sb[:, j:j + 1])
                se = kpool.tile([128, 64], F32, tag="se", name=f"se{j}")
                nc.scalar.activation(se[:], gp_f[:, j * C:(j + 1) * C],
                                     AF.Exp, bias=bt2_sb[:, 4 + j:5 + j])
                nc.scalar.activation(so[:, j * C:(j + 1) * C], se[:],
                                     AF.Ln, bias=1.0)
            nc.sync.dma_start(mean_d[:].transpose([1, 0, 2]),
                              mo[:].rearrange("p (k n) -> p k n", k=KL))
            nc.sync.dma_start(std_d[:].transpose([1, 0, 2]),
                              so[:].rearrange("p (k n) -> p k n", k=KL))

    if legalize:
        _legalize_waits(nc)
    return nc


V3_INPUTS = ["dataT", "dtm", "wo1", "wo2", "whh", "wih", "bg",
             "wt1", "wt2", "bt1t", "bt2t", "eye128"]


# ============================ v1 build (fallback) =========================

def _build(bias_o_zero, legalize=True):
    _patch_tile_drain()
    nc = bass.Bass()
    g = lambda n, s, d: nc.dram_tensor(n, s, d, kind="ExternalInput").ap()
    dataT_d = g("dataT", [KD, 128, TB], BF16)
    dtm_d = g("dtm", [T, 128, 512], F32)
    wo1_d = g("wo1", [KL, KL, 128, 128], BF16)
    wo2_d = g("wo2", [KL, KL, 128, 128], BF16)
    whh_d = g("whh", [KL, JG, 128, 128], BF16)
    wih_d = g("wih", [KD, JG, 128, 128], BF16)
    bg_d = g("bg", [128, JG], F32)
    bo1_d = g("bo1t", [128, KL], F32)
    bo2_d = g("bo2t", [128, KL], F32)
    wt1_d = g("wt1", [KL, 128, 100], BF16)
    wt2_d = g("wt2", [8, 100, 128], BF16)
    bt1_d = g("bt1t", [128, 1], F32)
    bt2_d = g("bt2t", [128, 8], F32)
    mean_d = nc.dram_tensor("meanT", [KL, 128, BS], F32, kind="ExternalOutput").ap()
    std_d = nc.dram_tensor("stdT", [KL, 128, BS], F32, kind="ExternalOutput").ap()
    pre_d = nc.dram_tensor("pre_scratch", [JG, 128, TB], F32).ap()

    with tile.TileContext(nc) as tc:
        import contextlib
        with contextlib.ExitStack() as ctx:
            wpool = ctx.enter_context(tc.tile_pool(name="weights", bufs=1))
            spool = ctx.enter_context(tc.tile_pool(name="state", bufs=1))
            dpool = ctx.enter_context(tc.tile_pool(name="stream", bufs=3))
            kpool = ctx.enter_context(tc.tile_pool(name="work", bufs=2))

            data_sb = wpool.tile([128, KD * TB], BF16, tag="data")
            nc.sync.dma_start(
                data_sb[:].rearrange("p (k n) -> p k n", k=KD),
                dataT_d[:].transpose([1, 0, 2]))
            wih_sb = wpool.tile([128, KD * JG * 128], BF16, tag="wih")
            nc.sync.dma_start(
                wih_sb[:].rearrange("p (k j n) -> p k j n", k=KD, j=JG),
                wih_d[:].transpose([2, 0, 1, 3]))
            wo1_sb = wpool.tile([128, KL * KL * 128], BF16, tag="wo1")
            nc.sync.dma_start(
                wo1_sb[:].rearrange("p (k j n) -> p k j n", k=KL, j=KL),
                wo1_d[:].transpose([2, 0, 1, 3]))
            wo2_sb = wpool.tile([128, KL * KL * 128], BF16, tag="wo2")
            nc.sync.dma_start(
                wo2_sb[:].rearrange("p (k j n) -> p k j n", k=KL, j=KL),
                wo2_d[:].transpose([2, 0, 1, 3]))
            whh_sb = wpool.tile([128, KL * JG * 128], BF16, tag="whh")
            nc.sync.dma_start(
                whh_sb[:].rearrange("p (k j n) -> p k j n", k=KL, j=JG),
                whh_d[:].transpose([2, 0, 1, 3]))
            bg_sb = wpool.tile([128, JG], F32, tag="bg")
            nc.sync.dma_start(bg_sb[:], bg_d[:])
            bo1_sb = wpool.tile([128, KL], F32, tag="bo1")
            nc.sync.dma_start(bo1_sb[:], bo1_d[:])
            bo2_sb = wpool.tile([128, KL], F32, tag="bo2")
            nc.sync.dma_start(bo2_sb[:], bo2_d[:])
            wt1_sb = wpool.tile([128, KL * 100], BF16, tag="wt1")
            nc.sync.dma_start(
                wt1_sb[:].rearrange("p (k n) -> p k n", k=KL),
                wt1_d[:].transpose([1, 0, 2]))
            wt2_sb = wpool.tile([100, 8 * 128], BF16, tag="wt2")
            nc.sync.dma_start(
                wt2_sb[:].rearrange("p (j n) -> p j n", j=8),
                wt2_d[:].transpose([1, 0, 2]))
            bt1_sb = wpool.tile([128, 1], F32, tag="bt1")
            nc.sync.dma_start(bt1_sb[:], bt1_d[:])
            bt2_sb = wpool.tile([128, 8], F32, tag="bt2")
            nc.sync.dma_start(bt2_sb[:], bt2_d[:])

            wihv = lambda k, j: wih_sb[:, (k * JG + j) * 128:(k * JG + j + 1) * 128]
            whhv = lambda k, j: whh_sb[:, (k * JG + j) * 128:(k * JG + j + 1) * 128]
            wo1v = lambda k, j: wo1_sb[:, (k * KL + j) * 128:(k * KL + j + 1) * 128]
            wo2v = lambda k, j: wo2_sb[:, (k * KL + j) * 128:(k * KL + j + 1) * 128]
            datav = lambda k, sl: data_sb[:, k * TB + sl.start:k * TB + sl.stop]

            # ---- phase 1: bulk input-gate precompute -----------------------
            NB = min(512, TB)
            NBC = TB // NB
            GRP = min(4, NBC)
            bctx = tc.tile_pool(name="bulkps", bufs=2, space="PSUM")
            bpool = bctx.__enter__()
            for bcg in range((NBC + GRP - 1) // GRP):
                grp = min(GRP, NBC - bcg * GRP)
                pts = [bpool.tile([128, NB], F32, tag=f"bulk{i}",
                                  name=f"bulk{bcg}_{i}") for i in range(grp)]
                for j in range(JG):
                    for k in range(KD):
                        for i in range(grp):
                            bc = bcg * GRP + i
                            nc.tensor.matmul(
                                pts[i][:], wihv(k, j),
                                datav(k, slice(bc * NB, (bc + 1) * NB)),
                                start=(k == 0), stop=(k == KD - 1))
                    for i in range(grp):
                        bc = bcg * GRP + i
                        st = dpool.tile([128, NB], F32, tag="bulkstage")
                        if j % 2 == 0:
                            nc.scalar.activation(st[:], pts[i][:], AF.Identity,
                                                 bias=bg_sb[:, j:j + 1])
                        else:
                            nc.vector.tensor_scalar_add(st[:], pts[i][:],
                                                        bg_sb[:, j:j + 1])
                        nc.sync.dma_start(pre_d[j, :, bc * NB:(bc + 1) * NB], st[:])

            bctx.__exit__(None, None, None)

            # ---- phase 2: recurrence ---------------------------------------
            ppool = ctx.enter_context(tc.tile_pool(name="recps", bufs=2,
                                                   space="PSUM"))
            gpool = ppool
            xT = spool.tile([128, 256], F32, tag="xT")
            cT = spool.tile([128, 256], F32, tag="cT")
            xbf = spool.tile([128, 256], BF16, tag="xbf")
            nc.vector.memset(xT[:], 0.0)
            nc.vector.memset(cT[:], 0.0)
            nc.vector.memset(xbf[:], 0.0)

            for t in range(T):
                dtm_sb = dpool.tile([128, 512], F32, tag="dtm")
                nc.sync.dma_start(dtm_sb[:], dtm_d[t])
                pre_sb = dpool.tile([128, JG * C], F32, tag="pre")
                nc.sync.dma_start(
                    pre_sb[:].rearrange("p (j n) -> p j n", j=JG),
                    pre_d[:, :, t * C:(t + 1) * C].transpose([1, 0, 2]))

                u_ps = ppool.tile([128, 256], F32, tag="u")
                for j in range(KL):
                    for k in range(KL):
                        nc.tensor.matmul(u_ps[:, j * C:(j + 1) * C], wo1v(k, j),
                                         xbf[:, k * C:(k + 1) * C],
                                         start=(k == 0), stop=(k == KL - 1))
                t1_bf = kpool.tile([128, 256], BF16, tag="t1")
                if bias_o_zero:
                    nc.scalar.activation(t1_bf[:], u_ps[:], AF.Tanh)
                else:
                    for j in range(KL):
                        nc.scalar.activation(t1_bf[:, j * C:(j + 1) * C],
                                             u_ps[:, j * C:(j + 1) * C], AF.Tanh,
                                             bias=bo1_sb[:, j:j + 1])
                v_ps = ppool.tile([128, 256], F32, tag="v")
                for j in range(KL):
                    for k in range(KL):
                        nc.tensor.matmul(v_ps[:, j * C:(j + 1) * C], wo2v(k, j),
                                         t1_bf[:, k * C:(k + 1) * C],
                                         start=(k == 0), stop=(k == KL - 1))
                vd = kpool.tile([128, 256], F32, tag="vd")
                if bias_o_zero:
                    nc.vector.tensor_mul(vd[:], v_ps[:], dtm_sb[:, 0:256])
                else:
                    for j in range(KL):
                        nc.vector.scalar_tensor_tensor(
                            vd[:, j * C:(j + 1) * C], v_ps[:, j * C:(j + 1) * C],
                            bo2_sb[:, j:j + 1], dtm_sb[:, j * C:(j + 1) * C],
                            mybir.AluOpType.add, mybir.AluOpType.mult)
                nc.vector.tensor_add(xT[:], xT[:], vd[:])
                xobf = kpool.tile([128, 256], BF16, tag="xobf")
                nc.scalar.copy(xobf[:], xT[:])

                g_ps = gpool.tile([128, JG * C], F32, tag="g")
                for j in range(JG):
                    for k in range(KL):
                        nc.tensor.matmul(g_ps[:, j * C:(j + 1) * C], whhv(k, j),
                                         xobf[:, k * C:(k + 1) * C],
                                         start=(k == 0), stop=(k == KL - 1))
                nc.vector.tensor_add(g_ps[:], g_ps[:], pre_sb[:])

                sg = kpool.tile([128, 768], F32, tag="sg")
                nc.scalar.activation(sg[:], g_ps[:, 0:768], AF.Sigmoid)
                tg = kpool.tile([128, 256], F32, tag="tg")
                nc.scalar.activation(tg[:], g_ps[:, 768:1024], AF.Tanh)

                m1 = kpool.tile([128, 256], F32, tag="m1")
                nc.vector.tensor_mul(m1[:], sg[:, 256:512], cT[:])
                m2 = kpool.tile([128, 256], F32, tag="m2")
                nc.vector.tensor_mul(m2[:], sg[:, 0:256], tg[:])
                c2 = kpool.tile([128, 256], F32, tag="c2")
                nc.vector.tensor_add(c2[:], m1[:], m2[:])
                tc2 = kpool.tile([128, 256], F32, tag="tc2")
                nc.scalar.activation(tc2[:], c2[:], AF.Tanh)
                h = kpool.tile([128, 256], F32, tag="h")
                nc.vector.tensor_mul(h[:], sg[:, 512:768], tc2[:])

                mask_i = dtm_sb[:, 256:512].bitcast(dt.int32)
                nc.vector.copy_predicated(xT[:], mask_i, h[:])
                nc.vector.copy_predicated(cT[:], mask_i, c2[:])
                nc.scalar.copy(xbf[:], xT[:])

            # ---- phase 3: transform_z0 head --------------------------------
            q_ps = ppool.tile([128, C], F32, tag="u")
            for k in range(KL):
                nc.tensor.matmul(q_ps[0:100, :], wt1_sb[:, k * 100:(k + 1) * 100],
                                 xbf[:, k * C:(k + 1) * C],
                                 start=(k == 0), stop=(k == KL - 1))
            tq_bf = kpool.tile([100, C], BF16, tag="tq")
            nc.scalar.activation(tq_bf[:], q_ps[0:100, :], AF.Tanh,
                                 bias=bt1_sb[0:100, :])
            z_ps = ppool.tile([128, 8 * C], F32, tag="v")
            for j in range(8):
                nc.tensor.matmul(z_ps[:, j * C:(j + 1) * C],
                                 wt2_sb[:, j * 128:(j + 1) * 128], tq_bf[:],
                                 start=True, stop=True)
            mo = kpool.tile([128, 256], F32, tag="mo")
            so = kpool.tile([128, 256], F32, tag="so")
            for j in range(KL):
                nc.scalar.activation(mo[:, j * C:(j + 1) * C],
                                     z_ps[:, j * C:(j + 1) * C], AF.Identity,
                                     bias=bt2_sb[:, j:j + 1])
                se = kpool.tile([128, 64], F32, tag="se", name=f"se{j}")
                nc.scalar.activation(se[:], z_ps[:, (4 + j) * C:(5 + j) * C],
                                     AF.Exp, bias=bt2_sb[:, 4 + j:5 + j])
                nc.scalar.activation(so[:, j * C:(j + 1) * C], se[:],
                                     AF.Ln, bias=1.0)
            nc.sync.dma_start(mean_d[:].transpose([1, 0, 2]),
                              mo[:].rearrange("p (k n) -> p k n", k=KL))
            nc.sync.dma_start(std_d[:].transpose([1, 0, 2]),
                              so[:].rearrange("p (k n) -> p k n", k=KL))

    if legalize:
        _legalize_waits(nc)
    return nc


# ============================ host side ===================================

_CACHE = {}
_last_in_maps = None


def _prepare_inmaps(truth, mask, time_steps, W_ih, W_hh, b_ih, b_hh,
                    Wt1, bt1, Wt2, bt2, Wo1, bo1, Wo2, bo2):
    # gate order [i, f, o, g]
    perm = np.concatenate([np.arange(0, 512), np.arange(512, 1024),
                           np.arange(1536, 2048), np.arange(1024, 1536)])
    W_ih_p, W_hh_p = W_ih[perm], W_hh[perm]
    b_p = (b_ih + b_hh)[perm]
    tile4 = lambda w, kk, jj: np.ascontiguousarray(
        w.T.reshape(kk, 128, jj, 128).transpose(0, 2, 1, 3)).astype(BF)
    wo1_t = tile4(Wo1, KL, KL)
    wo2_t = tile4(Wo2, KL, KL)
    whh_t = tile4(W_hh_p, KL, JG)
    wih_t = tile4(W_ih_p, KD, JG)
    bg_t = np.ascontiguousarray(b_p.reshape(JG, 128).T)
    bo1_t = np.ascontiguousarray(bo1.reshape(KL, 128).T)
    bo2_t = np.ascontiguousarray(bo2.reshape(KL, 128).T)
    wt1_t = np.ascontiguousarray(Wt1.T.reshape(KL, 128, 100)).astype(BF)
    wt2_t = np.ascontiguousarray(
        Wt2.T.reshape(100, 8, 128).transpose(1, 0, 2)).astype(BF)
    bt1_t = np.zeros((128, 1), np.float32)
    bt1_t[:100, 0] = bt1
    bt2_t = np.ascontiguousarray(bt2.reshape(8, 128).T)

    dt_rest = time_steps[:, :-1] - time_steps[:, 1:]
    dt_full = np.concatenate(
        [np.full((B, 1), -0.01, np.float32), dt_rest[:, ::-1]], axis=1)  # [B,T]

    in_maps = []
    for c in range(NCORES):
        sh = slice(c * BS, (c + 1) * BS)
        data = np.concatenate([truth[sh], mask[sh]], axis=-1)   # [BS,T,2D]
        xs = np.ascontiguousarray(data[:, ::-1, :].transpose(1, 0, 2))
        dataT = np.ascontiguousarray(
            xs.transpose(2, 0, 1).reshape(2 * D, TB))           # [2D, T*BS]
        dataT_t = dataT.reshape(KD, 128, TB).astype(BF)
        dts = dt_full[sh].T                                      # [T,BS]
        mseq = (xs[:, :, D:].sum(-1) > 0).astype(np.float32)     # [T,BS]
        dtm = np.empty((T, 128, 512), np.float32)
        dtm[:, :, 0:256] = np.tile(dts[:, None, :], (1, 128, 4))
        dtm[:, :, 256:512] = np.tile(mseq[:, None, :], (1, 128, 4))
        in_maps.append({
            "dataT": dataT_t, "dtm": dtm,
            "wo1": wo1_t, "wo2": wo2_t, "whh": whh_t, "wih": wih_t,
            "bg": bg_t, "bo1t": bo1_t, "bo2t": bo2_t,
            "wt1": wt1_t, "wt2": wt2_t, "bt1t": bt1_t, "bt2t": bt2_t,
            "eye128": np.eye(128, dtype=np.float32).astype(BF),
        })

    return in_maps


def kernel(truth, mask, time_steps, W_ih, W_hh, b_ih, b_hh,
           Wt1, bt1, Wt2, bt2, Wo1, bo1, Wo2, bo2):
    import os
    truth = np.asarray(truth, np.float32)
    mask = np.asarray(mask, np.float32)
    time_steps = np.asarray(time_steps, np.float32)
    W_ih, W_hh = np.asarray(W_ih, np.float32), np.asarray(W_hh, np.float32)
    b_ih, b_hh = np.asarray(b_ih, np.float32), np.asarray(b_hh, np.float32)
    Wt1, bt1 = np.asarray(Wt1, np.float32), np.asarray(bt1, np.float32)
    Wt2, bt2 = np.asarray(Wt2, np.float32), np.asarray(bt2, np.float32)
    Wo1, bo1 = np.asarray(Wo1, np.float32), np.asarray(bo1, np.float32)
    Wo2, bo2 = np.asarray(Wo2, np.float32), np.asarray(bo2, np.float32)

    bias_o_zero = not (np.any(bo1) or np.any(bo2))
    all_observed = bool((mask.sum(-1) > 0).all())
    use_v3 = (bias_o_zero and all_observed
              and os.environ.get("KERNEL_V1", "0") != "1")

    in_maps = _prepare_inmaps(truth, mask, time_steps, W_ih, W_hh, b_ih, b_hh,
                              Wt1, bt1, Wt2, bt2, Wo1, bo1, Wo2, bo2)
    global _last_in_maps
    if use_v3:
        key = ("odernn_v3",)
        if key not in _CACHE:
            _CACHE[key] = _build_v3()
        nc = _CACHE[key]
        in_maps = [{k: m[k] for k in V3_INPUTS} for m in in_maps]
    else:
        key = ("odernn", bias_o_zero)
        if key not in _CACHE:
            _CACHE[key] = _build(bias_o_zero)
        nc = _CACHE[key]
    _last_in_maps = in_maps

    res = run_bass_kernel_spmd(nc, in_maps, list(range(NCORES))).results

    mean = np.empty((1, B, Z0), np.float32)
    std = np.empty((1, B, Z0), np.float32)
    for c in range(NCORES):
        mt, st = res[c]["meanT"], res[c]["stdT"]                 # [KL,128,BS]
        mean[0, c * BS:(c + 1) * BS] = mt.reshape(Z0, BS).T
        std[0, c * BS:(c + 1) * BS] = st.reshape(Z0, BS).T
    return mean, std
